# revision 10
# baseline (speedup 1.0000x reference)
"""Trainium2 Bass kernel for nn_EnhancedFractionalPINO.

Math folding (all precomputed on host, per call):
  reference out = iDFT( relu(relu(relu(GLconv(DFT(x))@Ws1+b1) @ (Ws2@Wn1)
                  + (bs2@Wn1+bn1)) @ Wn2 + bn2) @ Wn3 + bn3 )
  - Ws2@Wn1 folds to a single 512x512 matrix U (no relu between the two
    12288-wide matmuls in the reference), eliminating both of them.
  - The GL fractional conv (lower-triangular Toeplitz T0 within a batch row
    plus a 512-sample halo from the previous batch) and the forward 2-D DFT
    fold into Ws1:  V0_pix = D^T T0^T Ws1 acts on raw pixels;  a 512x512
    V1h acts on the last 512 DFT values of the previous batch's channel-2
    image (computed on host via fft2).  Full in-batch GL taps.
  - The inverse 2-D DFT folds into Wn3: W5f = Wn3 o blockdiag(Re(iDFT)).
  - The final bias (b5f = bn3 o iDFT) is added on the host (linear).

Precision: L1 weights are float8-e3m4 with a per-chunk pow2 scale compensated
exactly in the disjoint x / halo column groups; the last K-half of W5f is
e3m4 with a pow2 scale compensated in h2's fb=2,3 blocks (relu commutes with
positive scales).  All other tensors fp16; PSUM accumulation fp32.

Per core (batch-parallel, 32 batches/core): a 4-layer MLP
  h   = relu([halo | x_pixels] @ [V1h; V0_pix] + b1)      (K=12800 streamed)
  h1  = relu(h @ U + cU);  h2 = relu(h1 @ W4 + b4)        (weights resident)
  out = h2 @ W5f                                          (N=12288 streamed)
"""

import numpy as np

import concourse.bass as bass
import concourse.mybir as mybir
import concourse.tile as tile
from concourse import bacc
from concourse.bass_utils import run_bass_kernel_spmd

F32 = mybir.dt.float32
F16 = mybir.dt.float16
F8 = mybir.dt.float8e3
AF = mybir.ActivationFunctionType

B, C, H, W = 256, 3, 64, 64
MODES = C * H * W              # 12288
NTOT = B * MODES
ALPHA = 0.5
NCORE = 8
BS = B // NCORE                # 32 batches per core
XCOLS = 33 * 96 + 128          # 3072 pixel chunks + view pad + 128 halo cols

LAM_H, LAM_1, LAM_2 = 16.0, 4.0, 4.0
K5 = 11                        # pow2 scale exponent for the fp8 block of W5f


# ---------------------------------------------------------------- host folds
def _fold_weights(Ws1, bs1, Ws2, bs2, Wn1, bn1, Wn2, bn2, Wn3, bn3):
    f16 = lambda a: np.ascontiguousarray(a, dtype=np.float16)
    f83 = mybir.dt.np(F8)
    s = float(np.float64(1.0 / (NTOT - 1)) ** (-ALPHA))

    # GL weights w_j (enough taps for in-batch + 512-halo reach)
    j = np.arange(1, 13312, dtype=np.float64)
    wgl = np.concatenate([[1.0], np.cumprod((j - 1.0 - ALPHA) / j)])

    # V0[m] = sum_d w_d W1s[m+d];  V1h[m'] = sum_k w_{k+512-m'} W1s[k]
    L = 32768
    W1s = Ws1.astype(np.float64) * (s / LAM_H)
    corr = np.fft.irfft(
        np.fft.rfft(W1s, n=L, axis=0) * np.conj(np.fft.rfft(wgl, n=L))[:, None],
        n=L, axis=0)
    V0 = corr[:MODES].astype(np.float32)
    V1h = corr[L - 512:].astype(np.float32)

    jk = np.outer(np.arange(64), np.arange(64)).astype(np.float64)
    Cm = np.cos(2 * np.pi * jk / 64).astype(np.float32)
    Sm = np.sin(2 * np.pi * jk / 64).astype(np.float32)

    # V0_pix[(y,z),n] = sum_{u,v} (C[u,y]C[v,z] - S[u,y]S[v,z]) V0[(u,v),n]
    V0c = V0.reshape(3, 64, 64, 512)
    V0_pix = (np.einsum('uy,cuvn,vz->cyzn', Cm, V0c, Cm, optimize=True)
              - np.einsum('uy,cuvn,vz->cyzn', Sm, V0c, Sm, optimize=True)
              ).reshape(MODES, 512)
    Vcat = np.concatenate([V1h, V0_pix], axis=0)            # (12800, 512)

    # e3m4 per-chunk pow2 scaling; the scale is compensated exactly in the
    # (disjoint) x / halo column groups.
    am = np.abs(Vcat.reshape(100, 128 * 512)).max(axis=1)
    kq = np.clip(np.floor(np.log2(15.5 / np.maximum(am, 1e-12))), -12, 12)
    Vq8 = (Vcat.reshape(100, 128, 512)
           * (2.0 ** kq)[:, None, None].astype(np.float32)).astype(f83)
    xscale = (2.0 ** (-kq)).astype(np.float32)

    U = (Ws2.astype(np.float32) @ Wn1.astype(np.float32)) * np.float32(LAM_H / LAM_1)
    cU = ((bs2.astype(np.float32) @ Wn1.astype(np.float32) + bn1)
          / np.float32(LAM_1))
    W4 = Wn2 * np.float32(LAM_1 / LAM_2)

    # W5f = (Wn3 o Re(iDFT)) * LAM_2 ; b5f = bn3 o Re(iDFT)  (host-added)
    W5c = Wn3.astype(np.float32).reshape(512, 3, 64, 64)
    W5f = ((np.einsum('rcuv,uy,vz->rcyz', W5c, Cm, Cm, optimize=True)
            - np.einsum('rcuv,uy,vz->rcyz', W5c, Sm, Sm, optimize=True))
           * np.float32(LAM_2 / 4096.0)).reshape(512, MODES)
    b5c = bn3.astype(np.float32).reshape(3, 64, 64)
    b5f = ((np.einsum('cuv,uy,vz->cyz', b5c, Cm, Cm, optimize=True)
            - np.einsum('cuv,uy,vz->cyz', b5c, Sm, Sm, optimize=True))
           / np.float32(4096.0)).reshape(MODES)

    # split W5f: K-blocks fb=0,1 fp16, fb=2,3 e3m4 scaled by 2^K5
    # (compensated via h2's fb=2,3 blocks, scaled 2^-K5 at the L4 activation)
    W5r = W5f.reshape(4, 128, 12, 1024)
    lim = np.float32(15.5 * 2.0 ** (-K5))
    W5r = np.concatenate([W5r[0:2], np.clip(W5r[2:4], -lim, lim)])
    b4t = (bn2 / LAM_2).reshape(4, 128).T.astype(np.float32).copy()
    b4t[:, 2:4] *= np.float32(2.0 ** (-K5))

    return {
        "w1f": np.ascontiguousarray(
            Vq8.reshape(25, 4, 128, 512).transpose(0, 2, 1, 3)),
        "_xscale": xscale,
        "_b5f": b5f,
        "uw": f16(np.concatenate(
            [U.reshape(4, 128, 4, 128).transpose(2, 1, 0, 3)
              .reshape(4, 128, 512).transpose(1, 0, 2),
             W4.reshape(4, 128, 4, 128).transpose(2, 1, 0, 3)
              .reshape(4, 128, 512).transpose(1, 0, 2)], axis=1)),
        "w5f16": f16(W5r[0:2].transpose(2, 1, 0, 3)),
        "w5f8": np.ascontiguousarray(
            (W5r[2:4] * np.float32(2.0 ** K5)).transpose(2, 1, 0, 3)
            .astype(f83)),
        "b1t": np.ascontiguousarray((bs1 / LAM_H).reshape(4, 128).T,
                                    dtype=np.float32),
        "cUt": np.ascontiguousarray(cU.reshape(4, 128).T, dtype=np.float32),
        "b4t": np.ascontiguousarray(b4t),
    }


# ---------------------------------------------------------------- bass module
_NC_CACHE = None


def _build_nc():
    nc = bacc.Bacc("TRN2", target_bir_lowering=False, debug=False,
                   num_devices=NCORE)

    def din(name, shape, dt=F16):
        return nc.dram_tensor(name, shape, dt, kind="ExternalInput")

    d_xpix = din("xpix", (128, XCOLS))
    d_w1f = din("w1f", (25, 128, 4, 512), F8)
    d_uw = din("uw", (128, 8, 512))
    d_w5f16 = din("w5f16", (12, 128, 2, 1024))
    d_w5f8 = din("w5f8", (12, 128, 2, 1024), F8)
    d_b1 = nc.dram_tensor("b1t", (128, 4), F32, kind="ExternalInput")
    d_cU = nc.dram_tensor("cUt", (128, 4), F32, kind="ExternalInput")
    d_b4 = nc.dram_tensor("b4t", (128, 4), F32, kind="ExternalInput")
    d_out = nc.dram_tensor("out", (BS, MODES), F16, kind="ExternalOutput")

    with tile.TileContext(nc) as tc:
        with tc.tile_pool(name="cpool", bufs=1) as cpool, \
             tc.tile_pool(name="wp", bufs=8) as wp, \
             tc.tile_pool(name="wp5", bufs=6) as wp5, \
             tc.tile_pool(name="sp5", bufs=8) as sp5:
            xpix = cpool.tile([128, XCOLS], F16, tag="xpix")
            b1t = cpool.tile([128, 4], F32, tag="b1t")
            cUs = cpool.tile([128, 4], F32, tag="cUs")
            b4s = cpool.tile([128, 4], F32, tag="b4s")
            uw = cpool.tile([128, 8, 512], F16, tag="uw")
            hT = cpool.tile([128, 4, 32], F16, tag="hT")
            h1T = cpool.tile([128, 4, 32], F16, tag="h1T")
            h2T = cpool.tile([128, 4, 32], F16, tag="h2T")

            # big stream on sync; small consts on gpsimd (SWDGE: no HWDGE
            # contention with the stream's descriptor generation)
            nc.sync.dma_start(xpix[:], d_xpix[:])
            for t, dref in ((b1t, d_b1), (cUs, d_cU), (b4s, d_b4)):
                nc.scalar.dma_start(t[:], dref[:])

            vx = xpix[:, 0:3168].rearrange("p (b k) -> p b k", b=33)
            vh = xpix[:, 3168:3296].rearrange("p (b k) -> p b k", b=32)

            # ======= L1: hT = relu(W^T [halo|x]^T + b1), computed directly
            # in transposed form: weights are the stationary operand (out ap
            # is only 32), so no transposes and a per-partition bias.
            with tc.tile_pool(name="ps1", bufs=1, space="PSUM") as ps1:
                accs = [ps1.tile([128, 32], F32, tag=f"acc{nb}",
                                 name=f"acc{nb}") for nb in range(4)]
                for K4 in range(25):
                    wt = wp.tile([128, 4, 512], F8, tag="wt")
                    nc.sync.dma_start(wt[:], d_w1f[K4])
                    for jj in range(4):
                        q = 4 * K4 + jj
                        src = vh[:, :, q] if q < 4 else vx[:, 0:32, q - 4]
                        for nb in range(4):
                            nc.tensor.matmul(
                                accs[nb][:],
                                wt[:, jj, nb * 128:(nb + 1) * 128], src,
                                start=(q == 0), stop=(q == 99))
                # uw rides the stream right after w1f (inside the pool scope
                # so no released-zone barrier blocks it)
                nc.sync.dma_start(uw[:], d_uw[:])
                for nb in range(4):
                    nc.scalar.activation(hT[:, nb, :], accs[nb][:], AF.Relu,
                                         bias=b1t[:, nb:nb + 1])

            # ======= L2 (U) and L4 (W4): weights-stationary 512->512 ========
            # h2's fb=3 block carries the 2^-K5 compensation for the fp8
            # quarter of W5f (relu commutes; b4t col 3 is pre-scaled).
            with tc.tile_pool(name="ps2", bufs=4, space="PSUM") as ps2:
                for wo, bias, src, dst in ((0, cUs, hT, h1T),
                                           (4, b4s, h1T, h2T)):
                    for f2b in range(4):
                        acc2 = ps2.tile([128, 32], F32, tag="acc2")
                        for fb in range(4):
                            nc.tensor.matmul(
                                acc2[:],
                                uw[:, wo + f2b, fb * 128:(fb + 1) * 128],
                                src[:, fb, :], start=(fb == 0), stop=(fb == 3))
                        scl = (2.0 ** (-K5)) if (wo == 4 and f2b >= 2) else 1.0
                        nc.scalar.activation(dst[:, f2b, :], acc2[:], AF.Relu,
                                             bias=bias[:, f2b:f2b + 1],
                                             scale=scl)

            # ======= L5: out = h2 @ W5f  (bias added on host) ===============
            with tc.tile_pool(name="ps5", bufs=6, space="PSUM") as ps5, \
                 tc.tile_pool(name="psq", bufs=2, space="PSUM") as psq:
                for mc2 in range(12):
                    wt16 = wp5.tile([128, 2, 1024], F16, tag="w5a")
                    wt8 = wp5.tile([128, 2, 1024], F8, tag="w5b")
                    if mc2 < 11:
                        nc.sync.dma_start(wt16[:], d_w5f16[mc2])
                        nc.sync.dma_start(wt8[:], d_w5f8[mc2])
                    else:
                        nc.sync.dma_start(wt16[:, :, 0:512],
                                          d_w5f16[mc2][:, :, 0:512])
                        nc.sync.dma_start(wt8[:], d_w5f8[mc2])
                        nc.sync.dma_start(wt16[:, :, 512:768],
                                          d_w5f16[mc2][:, :, 512:768])
                        nc.sync.dma_start(wt16[:, :, 768:1024],
                                          d_w5f16[mc2][:, :, 768:1024])
                    if mc2 < 11:
                        for half in range(2):
                            acc5 = ps5.tile([32, 512], F32, tag="acc5")
                            sl = slice(half * 512, (half + 1) * 512)
                            for fb in range(2):
                                nc.tensor.matmul(
                                    acc5[:], h2T[:, fb, :], wt16[:, fb, sl],
                                    start=(fb == 0), stop=False)
                            for fb in range(2):
                                nc.tensor.matmul(
                                    acc5[:], h2T[:, 2 + fb, :],
                                    wt8[:, fb, sl],
                                    start=False, stop=(fb == 1))
                            if half == 0:
                                osb = sp5.tile([32, 1024], F16, tag="osb")
                                nc.scalar.copy(osb[:, 0:512], acc5[:])
                            else:
                                nc.vector.tensor_copy(osb[:, 512:1024],
                                                      acc5[:])
                                nc.gpsimd.dma_start(
                                    d_out[:, mc2 * 1024:(mc2 + 1) * 1024],
                                    osb[:])
                    else:
                        # final group: chunk 22 as usual; chunk 23 in two
                        # 256-col sub-accs with the (early-arriving) fp8
                        # matmuls first, so only two ap-256 matmuls, a small
                        # copy and a small sync-queue writeback trail the last
                        # stream byte.
                        acc5 = ps5.tile([32, 512], F32, tag="acc5")
                        for fb in range(2):
                            nc.tensor.matmul(acc5[:], h2T[:, fb, :],
                                             wt16[:, fb, 0:512],
                                             start=(fb == 0), stop=False)
                        for fb in range(2):
                            nc.tensor.matmul(acc5[:], h2T[:, 2 + fb, :],
                                             wt8[:, fb, 0:512],
                                             start=False, stop=(fb == 1))
                        osb = sp5.tile([32, 1024], F16, tag="osb")
                        nc.scalar.copy(osb[:, 0:512], acc5[:])
                        for sub in range(2):
                            c0 = 512 + sub * 256
                            sq = slice(c0, c0 + 256)
                            accq = psq.tile([32, 256], F32, tag="accq")
                            for fb in range(2):
                                nc.tensor.matmul(accq[:], h2T[:, 2 + fb, :],
                                                 wt8[:, fb, sq],
                                                 start=(fb == 0), stop=False)
                            for fb in range(2):
                                nc.tensor.matmul(
                                    accq[:], h2T[:, fb, :],
                                    wt16[:, fb, sq],
                                    start=False, stop=(fb == 1))
                            if sub == 0:
                                nc.vector.tensor_copy(osb[:, 512:768],
                                                      accq[:])
                                nc.gpsimd.dma_start(
                                    d_out[:, mc2 * 1024:mc2 * 1024 + 768],
                                    osb[:, 0:768])
                            else:
                                nc.vector.tensor_copy(osb[:, 768:1024],
                                                      accq[:])
                                nc.sync.dma_start(
                                    d_out[:, mc2 * 1024 + 768:
                                          (mc2 + 1) * 1024],
                                    osb[:, 768:1024])

    nc.compile()
    return nc


def _get_nc():
    global _NC_CACHE
    if _NC_CACHE is None:
        _NC_CACHE = _build_nc()
    return _NC_CACHE


def _make_in_maps(x, Ws1, bs1, Ws2, bs2, Wn1, bn1, Wn2, bn2, Wn3, bn3):
    shared = _fold_weights(Ws1, bs1, Ws2, bs2, Wn1, bn1, Wn2, bn2, Wn3, bn3)
    xscale = shared.pop("_xscale")
    b5f = shared.pop("_b5f")

    # halo: last 512 DFT-real values of every channel-2 image
    hg_all = np.real(np.fft.fft2(x[:, 2]))[:, 56:64, :].reshape(B, 512)
    hg_all = (hg_all.reshape(B, 4, 128)
              * xscale[0:4][None, :, None]).astype(np.float16)

    in_maps = []
    for g in range(NCORE):
        xc = (x[g * BS:(g + 1) * BS].reshape(BS, 96, 128)
              * xscale[None, 4:, None]).astype(np.float16)
        xpix = np.zeros((128, XCOLS), np.float16)
        xpix[:, :BS * 96] = xc.reshape(BS * 96, 128).T
        for b in range(BS):
            gi = g * BS + b - 1
            if gi >= 0:
                xpix[:, 3168 + 4 * b:3168 + 4 * b + 4] = hg_all[gi].T
        in_maps.append({"xpix": np.ascontiguousarray(xpix), **shared})
    return in_maps, b5f


def kernel(**inputs):
    x = np.ascontiguousarray(inputs["x"], dtype=np.float32)
    nc = _get_nc()
    in_maps, b5f = _make_in_maps(
        x, inputs["Ws1"], inputs["bs1"], inputs["Ws2"], inputs["bs2"],
        inputs["Wn1"], inputs["bn1"], inputs["Wn2"], inputs["bn2"],
        inputs["Wn3"], inputs["bn3"])
    res = run_bass_kernel_spmd(nc, in_maps, list(range(NCORE)))
    out = np.empty((B, C, H, W), np.float32)
    for g in range(NCORE):
        out[g * BS:(g + 1) * BS] = (
            (res.results[g]["out"].astype(np.float32) + b5f)
            .reshape(BS, C, H, W))
    return out


# revision 11
# speedup vs baseline: 1.0031x; 1.0031x over previous
"""Trainium2 Bass kernel for nn_EnhancedFractionalPINO.

Math folding (all precomputed on host, per call):
  reference out = iDFT( relu(relu(relu(GLconv(DFT(x))@Ws1+b1) @ (Ws2@Wn1)
                  + (bs2@Wn1+bn1)) @ Wn2 + bn2) @ Wn3 + bn3 )
  - Ws2@Wn1 folds to a single 512x512 matrix U (no relu between the two
    12288-wide matmuls in the reference), eliminating both of them.
  - The GL fractional conv (lower-triangular Toeplitz T0 within a batch row
    plus a 512-sample halo from the previous batch) and the forward 2-D DFT
    fold into Ws1:  V0_pix = D^T T0^T Ws1 acts on raw pixels;  a 512x512
    V1h acts on the last 512 DFT values of the previous batch's channel-2
    image (computed on host via fft2).  Full in-batch GL taps.
  - The inverse 2-D DFT folds into Wn3: W5f = Wn3 o blockdiag(Re(iDFT)).
  - The final bias (b5f = bn3 o iDFT) is added on the host (linear).

Precision: L1 weights are float8-e3m4 with a per-chunk pow2 scale compensated
exactly in the disjoint x / halo column groups; the last K-half of W5f is
e3m4 with a pow2 scale compensated in h2's fb=2,3 blocks (relu commutes with
positive scales).  All other tensors fp16; PSUM accumulation fp32.

Per core (batch-parallel, 32 batches/core): a 4-layer MLP
  h   = relu([halo | x_pixels] @ [V1h; V0_pix] + b1)      (K=12800 streamed)
  h1  = relu(h @ U + cU);  h2 = relu(h1 @ W4 + b4)        (weights resident)
  out = h2 @ W5f                                          (N=12288 streamed)
"""

import numpy as np

import concourse.bass as bass
import concourse.mybir as mybir
import concourse.tile as tile
from concourse import bacc
from concourse.bass_utils import run_bass_kernel_spmd

F32 = mybir.dt.float32
F16 = mybir.dt.float16
F8 = mybir.dt.float8e3
AF = mybir.ActivationFunctionType

B, C, H, W = 256, 3, 64, 64
MODES = C * H * W              # 12288
NTOT = B * MODES
ALPHA = 0.5
NCORE = 8
BS = B // NCORE                # 32 batches per core
XCOLS = 3200                   # 3072 pixel chunks + 128 halo cols (the
                               # 33-batch view pad overlaps the halo range)

LAM_H, LAM_1, LAM_2 = 16.0, 4.0, 4.0
K5 = 11                        # pow2 scale exponent for the fp8 block of W5f


# ---------------------------------------------------------------- host folds
def _fold_weights(Ws1, bs1, Ws2, bs2, Wn1, bn1, Wn2, bn2, Wn3, bn3):
    f16 = lambda a: np.ascontiguousarray(a, dtype=np.float16)
    f83 = mybir.dt.np(F8)
    s = float(np.float64(1.0 / (NTOT - 1)) ** (-ALPHA))

    # GL weights w_j (enough taps for in-batch + 512-halo reach)
    j = np.arange(1, 13312, dtype=np.float64)
    wgl = np.concatenate([[1.0], np.cumprod((j - 1.0 - ALPHA) / j)])

    # V0[m] = sum_d w_d W1s[m+d];  V1h[m'] = sum_k w_{k+512-m'} W1s[k]
    L = 32768
    W1s = Ws1.astype(np.float64) * (s / LAM_H)
    corr = np.fft.irfft(
        np.fft.rfft(W1s, n=L, axis=0) * np.conj(np.fft.rfft(wgl, n=L))[:, None],
        n=L, axis=0)
    V0 = corr[:MODES].astype(np.float32)
    V1h = corr[L - 512:].astype(np.float32)

    jk = np.outer(np.arange(64), np.arange(64)).astype(np.float64)
    Cm = np.cos(2 * np.pi * jk / 64).astype(np.float32)
    Sm = np.sin(2 * np.pi * jk / 64).astype(np.float32)

    # V0_pix[(y,z),n] = sum_{u,v} (C[u,y]C[v,z] - S[u,y]S[v,z]) V0[(u,v),n]
    V0c = V0.reshape(3, 64, 64, 512)
    V0_pix = (np.einsum('uy,cuvn,vz->cyzn', Cm, V0c, Cm, optimize=True)
              - np.einsum('uy,cuvn,vz->cyzn', Sm, V0c, Sm, optimize=True)
              ).reshape(MODES, 512)
    Vcat = np.concatenate([V1h, V0_pix], axis=0)            # (12800, 512)

    # e3m4 per-chunk pow2 scaling; the scale is compensated exactly in the
    # (disjoint) x / halo column groups.
    am = np.abs(Vcat.reshape(100, 128 * 512)).max(axis=1)
    kq = np.clip(np.floor(np.log2(15.5 / np.maximum(am, 1e-12))), -12, 12)
    Vq8 = (Vcat.reshape(100, 128, 512)
           * (2.0 ** kq)[:, None, None].astype(np.float32)).astype(f83)
    xscale = (2.0 ** (-kq)).astype(np.float32)

    U = (Ws2.astype(np.float32) @ Wn1.astype(np.float32)) * np.float32(LAM_H / LAM_1)
    cU = ((bs2.astype(np.float32) @ Wn1.astype(np.float32) + bn1)
          / np.float32(LAM_1))
    W4 = Wn2 * np.float32(LAM_1 / LAM_2)

    # W5f = (Wn3 o Re(iDFT)) * LAM_2 ; b5f = bn3 o Re(iDFT)  (host-added)
    W5c = Wn3.astype(np.float32).reshape(512, 3, 64, 64)
    W5f = ((np.einsum('rcuv,uy,vz->rcyz', W5c, Cm, Cm, optimize=True)
            - np.einsum('rcuv,uy,vz->rcyz', W5c, Sm, Sm, optimize=True))
           * np.float32(LAM_2 / 4096.0)).reshape(512, MODES)
    b5c = bn3.astype(np.float32).reshape(3, 64, 64)
    b5f = ((np.einsum('cuv,uy,vz->cyz', b5c, Cm, Cm, optimize=True)
            - np.einsum('cuv,uy,vz->cyz', b5c, Sm, Sm, optimize=True))
           / np.float32(4096.0)).reshape(MODES)

    # split W5f: K-blocks fb=0,1 fp16, fb=2,3 e3m4 scaled by 2^K5
    # (compensated via h2's fb=2,3 blocks, scaled 2^-K5 at the L4 activation)
    W5r = W5f.reshape(4, 128, 12, 1024)
    lim = np.float32(15.5 * 2.0 ** (-K5))
    W5r = np.concatenate([W5r[0:2], np.clip(W5r[2:4], -lim, lim)])
    b4t = (bn2 / LAM_2).reshape(4, 128).T.astype(np.float32).copy()
    b4t[:, 2:4] *= np.float32(2.0 ** (-K5))

    return {
        "w1f": np.ascontiguousarray(
            Vq8.reshape(25, 4, 128, 512).transpose(0, 2, 1, 3)),
        "_xscale": xscale,
        "_b5f": b5f,
        "uw": f16(np.concatenate(
            [U.reshape(4, 128, 4, 128).transpose(2, 1, 0, 3)
              .reshape(4, 128, 512).transpose(1, 0, 2),
             W4.reshape(4, 128, 4, 128).transpose(2, 1, 0, 3)
              .reshape(4, 128, 512).transpose(1, 0, 2)], axis=1)),
        "w5f16": f16(W5r[0:2].transpose(2, 1, 0, 3)),
        "w5f8": np.ascontiguousarray(
            (W5r[2:4] * np.float32(2.0 ** K5)).transpose(2, 1, 0, 3)
            .astype(f83)),
        "bcat": np.ascontiguousarray(np.concatenate(
            [(bs1 / LAM_H).reshape(4, 128).T.astype(np.float32),
             cU.reshape(4, 128).T.astype(np.float32), b4t], axis=1)),
    }


# ---------------------------------------------------------------- bass module
_NC_CACHE = None


def _build_nc():
    nc = bacc.Bacc("TRN2", target_bir_lowering=False, debug=False,
                   num_devices=NCORE)

    def din(name, shape, dt=F16):
        return nc.dram_tensor(name, shape, dt, kind="ExternalInput")

    d_xpix = din("xpix", (128, XCOLS))
    d_w1f = din("w1f", (25, 128, 4, 512), F8)
    d_uw = din("uw", (128, 8, 512))
    d_w5f16 = din("w5f16", (12, 128, 2, 1024))
    d_w5f8 = din("w5f8", (12, 128, 2, 1024), F8)
    d_bc = nc.dram_tensor("bcat", (128, 12), F32, kind="ExternalInput")
    d_out = nc.dram_tensor("out", (BS, MODES), F16, kind="ExternalOutput")

    with tile.TileContext(nc) as tc:
        with tc.tile_pool(name="cpool", bufs=1) as cpool, \
             tc.tile_pool(name="wp", bufs=8) as wp, \
             tc.tile_pool(name="wp5", bufs=6) as wp5, \
             tc.tile_pool(name="sp5", bufs=8) as sp5:
            xpix = cpool.tile([128, XCOLS], F16, tag="xpix")
            bcat = cpool.tile([128, 12], F32, tag="bcat")
            b1t, cUs, b4s = bcat[:, 0:4], bcat[:, 4:8], bcat[:, 8:12]
            uw = cpool.tile([128, 8, 512], F16, tag="uw")
            hT = cpool.tile([128, 4, 32], F16, tag="hT")
            h1T = cpool.tile([128, 4, 32], F16, tag="h1T")
            h2T = cpool.tile([128, 4, 32], F16, tag="h2T")

            # big stream on sync; small consts on gpsimd (SWDGE: no HWDGE
            # contention with the stream's descriptor generation)
            nc.sync.dma_start(xpix[:], d_xpix[:])
            nc.scalar.dma_start(bcat[:], d_bc[:])

            vx = xpix[:, 0:3168].rearrange("p (b k) -> p b k", b=33)
            vh = xpix[:, 3072:3200].rearrange("p (b k) -> p b k", b=32)

            # ======= L1: hT = relu(W^T [halo|x]^T + b1), computed directly
            # in transposed form: weights are the stationary operand (out ap
            # is only 32), so no transposes and a per-partition bias.
            with tc.tile_pool(name="ps1", bufs=1, space="PSUM") as ps1:
                accs = [ps1.tile([128, 32], F32, tag=f"acc{nb}",
                                 name=f"acc{nb}") for nb in range(4)]
                for K4 in range(25):
                    wt = wp.tile([128, 4, 512], F8, tag="wt")
                    nc.sync.dma_start(wt[:], d_w1f[K4])
                    for jj in range(4):
                        q = 4 * K4 + jj
                        src = vh[:, :, q] if q < 4 else vx[:, 0:32, q - 4]
                        for nb in range(4):
                            nc.tensor.matmul(
                                accs[nb][:],
                                wt[:, jj, nb * 128:(nb + 1) * 128], src,
                                start=(q == 0), stop=(q == 99))
                # uw rides the stream right after w1f (inside the pool scope
                # so no released-zone barrier blocks it)
                nc.sync.dma_start(uw[:], d_uw[:])
                for nb in range(4):
                    nc.scalar.activation(hT[:, nb, :], accs[nb][:], AF.Relu,
                                         bias=b1t[:, nb:nb + 1])

            # ======= L2 (U) and L4 (W4): weights-stationary 512->512 ========
            # h2's fb=3 block carries the 2^-K5 compensation for the fp8
            # quarter of W5f (relu commutes; b4t col 3 is pre-scaled).
            with tc.tile_pool(name="ps2", bufs=4, space="PSUM") as ps2:
                for wo, bias, src, dst in ((0, cUs, hT, h1T),
                                           (4, b4s, h1T, h2T)):
                    for f2b in range(4):
                        acc2 = ps2.tile([128, 32], F32, tag="acc2")
                        for fb in range(4):
                            nc.tensor.matmul(
                                acc2[:],
                                uw[:, wo + f2b, fb * 128:(fb + 1) * 128],
                                src[:, fb, :], start=(fb == 0), stop=(fb == 3))
                        scl = (2.0 ** (-K5)) if (wo == 4 and f2b >= 2) else 1.0
                        nc.scalar.activation(dst[:, f2b, :], acc2[:], AF.Relu,
                                             bias=bias[:, f2b:f2b + 1],
                                             scale=scl)

            # ======= L5: out = h2 @ W5f  (bias added on host) ===============
            with tc.tile_pool(name="ps5", bufs=6, space="PSUM") as ps5, \
                 tc.tile_pool(name="psq", bufs=2, space="PSUM") as psq:
                for mc2 in range(12):
                    wt16 = wp5.tile([128, 2, 1024], F16, tag="w5a")
                    wt8 = wp5.tile([128, 2, 1024], F8, tag="w5b")
                    if mc2 < 11:
                        nc.sync.dma_start(wt16[:], d_w5f16[mc2])
                        nc.sync.dma_start(wt8[:], d_w5f8[mc2])
                    else:
                        nc.sync.dma_start(wt16[:, :, 0:512],
                                          d_w5f16[mc2][:, :, 0:512])
                        nc.sync.dma_start(wt8[:], d_w5f8[mc2])
                        nc.sync.dma_start(wt16[:, :, 512:768],
                                          d_w5f16[mc2][:, :, 512:768])
                        nc.sync.dma_start(wt16[:, :, 768:1024],
                                          d_w5f16[mc2][:, :, 768:1024])
                    if mc2 < 11:
                        for half in range(2):
                            acc5 = ps5.tile([32, 512], F32, tag="acc5")
                            sl = slice(half * 512, (half + 1) * 512)
                            for fb in range(2):
                                nc.tensor.matmul(
                                    acc5[:], h2T[:, fb, :], wt16[:, fb, sl],
                                    start=(fb == 0), stop=False)
                            for fb in range(2):
                                nc.tensor.matmul(
                                    acc5[:], h2T[:, 2 + fb, :],
                                    wt8[:, fb, sl],
                                    start=False, stop=(fb == 1))
                            if half == 0:
                                osb = sp5.tile([32, 1024], F16, tag="osb")
                                nc.scalar.copy(osb[:, 0:512], acc5[:])
                            else:
                                nc.vector.tensor_copy(osb[:, 512:1024],
                                                      acc5[:])
                                nc.gpsimd.dma_start(
                                    d_out[:, mc2 * 1024:(mc2 + 1) * 1024],
                                    osb[:])
                    else:
                        # final group: chunk 22 as usual; chunk 23 in two
                        # 256-col sub-accs with the (early-arriving) fp8
                        # matmuls first, so only two ap-256 matmuls, a small
                        # copy and a small sync-queue writeback trail the last
                        # stream byte.
                        acc5 = ps5.tile([32, 512], F32, tag="acc5")
                        for fb in range(2):
                            nc.tensor.matmul(acc5[:], h2T[:, fb, :],
                                             wt16[:, fb, 0:512],
                                             start=(fb == 0), stop=False)
                        for fb in range(2):
                            nc.tensor.matmul(acc5[:], h2T[:, 2 + fb, :],
                                             wt8[:, fb, 0:512],
                                             start=False, stop=(fb == 1))
                        osb = sp5.tile([32, 1024], F16, tag="osb")
                        nc.scalar.copy(osb[:, 0:512], acc5[:])
                        for sub in range(2):
                            c0 = 512 + sub * 256
                            sq = slice(c0, c0 + 256)
                            accq = psq.tile([32, 256], F32, tag="accq")
                            for fb in range(2):
                                nc.tensor.matmul(accq[:], h2T[:, 2 + fb, :],
                                                 wt8[:, fb, sq],
                                                 start=(fb == 0), stop=False)
                            for fb in range(2):
                                nc.tensor.matmul(
                                    accq[:], h2T[:, fb, :],
                                    wt16[:, fb, sq],
                                    start=False, stop=(fb == 1))
                            if sub == 0:
                                nc.vector.tensor_copy(osb[:, 512:768],
                                                      accq[:])
                                nc.gpsimd.dma_start(
                                    d_out[:, mc2 * 1024:mc2 * 1024 + 768],
                                    osb[:, 0:768])
                            else:
                                nc.vector.tensor_copy(osb[:, 768:1024],
                                                      accq[:])
                                nc.sync.dma_start(
                                    d_out[:, mc2 * 1024 + 768:
                                          (mc2 + 1) * 1024],
                                    osb[:, 768:1024])

    nc.compile()
    return nc


def _get_nc():
    global _NC_CACHE
    if _NC_CACHE is None:
        _NC_CACHE = _build_nc()
    return _NC_CACHE


def _make_in_maps(x, Ws1, bs1, Ws2, bs2, Wn1, bn1, Wn2, bn2, Wn3, bn3):
    shared = _fold_weights(Ws1, bs1, Ws2, bs2, Wn1, bn1, Wn2, bn2, Wn3, bn3)
    xscale = shared.pop("_xscale")
    b5f = shared.pop("_b5f")

    # halo: last 512 DFT-real values of every channel-2 image
    hg_all = np.real(np.fft.fft2(x[:, 2]))[:, 56:64, :].reshape(B, 512)
    hg_all = (hg_all.reshape(B, 4, 128)
              * xscale[0:4][None, :, None]).astype(np.float16)

    in_maps = []
    for g in range(NCORE):
        xc = (x[g * BS:(g + 1) * BS].reshape(BS, 96, 128)
              * xscale[None, 4:, None]).astype(np.float16)
        xpix = np.zeros((128, XCOLS), np.float16)
        xpix[:, :BS * 96] = xc.reshape(BS * 96, 128).T
        for b in range(BS):
            gi = g * BS + b - 1
            if gi >= 0:
                xpix[:, 3072 + 4 * b:3072 + 4 * b + 4] = hg_all[gi].T
        in_maps.append({"xpix": np.ascontiguousarray(xpix), **shared})
    return in_maps, b5f


def kernel(**inputs):
    x = np.ascontiguousarray(inputs["x"], dtype=np.float32)
    nc = _get_nc()
    in_maps, b5f = _make_in_maps(
        x, inputs["Ws1"], inputs["bs1"], inputs["Ws2"], inputs["bs2"],
        inputs["Wn1"], inputs["bn1"], inputs["Wn2"], inputs["bn2"],
        inputs["Wn3"], inputs["bn3"])
    res = run_bass_kernel_spmd(nc, in_maps, list(range(NCORE)))
    out = np.empty((B, C, H, W), np.float32)
    for g in range(NCORE):
        out[g * BS:(g + 1) * BS] = (
            (res.results[g]["out"].astype(np.float32) + b5f)
            .reshape(BS, C, H, W))
    return out
